# revision 1
# baseline (speedup 1.0000x reference)
"""Trainium2 Bass kernel for nn_AudioMixer (4-track stereo mixer:
per-track 3-stage biquad EQ -> compressor -> Schroeder reverb on tracks 2,3
-> pan/volume mix -> limiter clip).

Sharding: core c = (track c//2, channel c%2) — each of the 8 cores processes
one full (track, channel) row of 1.44M samples end-to-end, then a
ReduceScatter over channel groups {0,2,4,6} / {1,3,5,7} sums the 4 weighted
tracks per channel; each core clips + writes a quarter of its channel.

Algorithms (validated against the jax reference):
 - EQ: combined per-track state-space cascade (order 6). Per 128-sample block:
   zero-state response via a lower-triangular Toeplitz matmul (bf16) on the
   PE; cross-block state corrections via two more matmuls whose rhs are
   stacked shifted "tail" rows. The correction add writes through a permuted
   AP view so that column c = 128w+b holds block 88b+w; the contiguous
   128x128 transposes of Phase C then directly produce the lane-major
   layout (lane p = samples [11264p, 11264(p+1))) with no DRAM round-trip.
 - Compressor: attack/release envelope via policy iteration in the
   u = env - lvl domain: u[n] = (d[n] - u[n-1]) * (-c[n]) with
   d[n] = lvl[n]-lvl[n-1], so each iteration is only mask -> affine -> scan
   (no per-iteration multiply). it0 (all-release) gets an exact cross-lane
   chain fix via REL-power decay; later iterations chain via current
   (Gauss-Seidel) boundary columns.
 - Reverb: comb y[n] = x[n] + fb*y[n-d] over "epochs" of d samples is a
   lower-triangular matmul over 128 stacked epochs (fb^(p-q)); tiles of 128
   epochs overlap by WEP warmup epochs. Same for both allpasses (the
   reference's first-epoch-zero quirk gets its own matrix for tile 0).
   All reverb DRAM scratch + matmuls in bf16; comb accumulation via
   bf16 accumulating DMAs.
"""
import math
from contextlib import ExitStack

import numpy as np

import concourse.bass as bass
import concourse.bacc as bacc
import concourse.mybir as mybir
import concourse.tile as tile
from concourse.bass_utils import run_bass_kernel_spmd

F32 = mybir.dt.float32
BF16 = mybir.dt.bfloat16

# ---------------------------------------------------------------- constants
SR = 48000
N = 1_440_000
NP = 128 * 11264          # padded row length (1441792)
F = 11264                 # per-lane length (128 lanes)
FCB = 2816                # EQ free-chunk (4 chunks); 2816 = 32*88
FC = 1408                 # compressor free-chunk (8 chunks)
CH = NP // 4              # ReduceScatter chunk per core = 360448

ATK = math.exp(-1.0 / (10.0 * 0.001 * SR))
REL = math.exp(-1.0 / (100.0 * 0.001 * SR))
THR = 10.0 ** (-18.0 / 20.0)
GR_EXP = 1.0 / 4.0 - 1.0
_BASE = int(SR * 0.03)
COMB_DELAYS = [_BASE, int(_BASE * 1.13), int(_BASE * 1.27), int(_BASE * 1.41)]
AP_DELAYS = [int(SR * 0.005), int(SR * 0.0017)]
FB = 0.3 + 0.5 * 0.6
WET = 0.3
CEIL = 10.0 ** (-1.0 / 20.0)

N_ITER = 3                # compressor policy iterations
TAILD = 6                 # tail rows for EQ correction
JMAX = 12                 # correction shift terms (zero-padded per track)
VEP = 104               # valid epochs per reverb tile
WEP = 24                # warmup epochs per tile

# ---------------------------------------------------------------- EQ host math
def _peak_coefs(freq, gain_db, q):
    A = 10.0 ** (gain_db / 40.0)
    w0 = 2.0 * math.pi * freq / SR
    al = math.sin(w0) / (2.0 * q)
    a0 = 1.0 + al / A
    return ((1.0 + al * A) / a0, -2.0 * math.cos(w0) / a0, (1.0 - al * A) / a0,
            -2.0 * math.cos(w0) / a0, (1.0 - al / A) / a0)

_IDENT = (1.0, 0.0, 0.0, 0.0, 0.0)
_PRESETS = {
    0: [(300.0, -3.0, 0.7), (3000.0, 3.0, 1.0), (8000.0, 2.0, 0.7)],
    1: [(80.0, 2.0, 0.7), (5000.0, 1.0, 1.0)],
    2: [(200.0, -2.0, 0.7), (6000.0, -1.0, 0.7)],
    3: [(1000.0, 2.0, 1.0)],
}

def _stage_coefs(track):
    bands = [_peak_coefs(*b) for b in _PRESETS[track]]
    bands += [_IDENT] * (3 - len(bands))
    return bands

def _biquad_ss(c):
    b0, b1, b2, a1, a2 = [float(v) for v in c]
    A = np.array([[-a1, 1.0], [-a2, 0.0]])
    B = np.array([[b1 - a1 * b0], [b2 - a2 * b0]])
    C = np.array([[1.0, 0.0]])
    D = np.array([[b0]])
    return A, B, C, D

def _cascade(ss_list):
    A1, B1, C1, D1 = ss_list[0]
    for A2, B2, C2, D2 in ss_list[1:]:
        n1, n2 = A1.shape[0], A2.shape[0]
        A = np.zeros((n1 + n2, n1 + n2))
        A[:n1, :n1] = A1
        A[n1:, :n1] = B2 @ C1
        A[n1:, n1:] = A2
        B = np.vstack([B1, B2 @ D1])
        C = np.hstack([D2 @ C1, C2])
        D = D2 @ D1
        A1, B1, C1, D1 = A, B, C, D
    return A1, B1, C1, D1

def _track_eq_consts(track, L=128):
    """Returns (T, zmT, zc):
    T    [L, L]   lower-triangular Toeplitz of the impulse response
    zmT  [L, n]   lhsT of the end-state matmul: z_b = M @ x_block,
                  M[:, t] = A^(L-1-t) B  (zmT[t, c] = M[c, t])
    zc   [n*JMAX, L]  correction lhsT: row (i', c) pairs with a z-stack row
                  holding z shifted by (JMAX - i') blocks; the correction for
                  block b is Phi @ s_{b-1} with s_{b-1} ~ sum_j A_L^(j-1)
                  z_{b-j}.  All entries O(1) -> bf16-safe (unlike the old
                  tail-probe fit whose inverse had entries ~500)."""
    A, B, C, D = _cascade([_biquad_ss(c) for c in _stage_coefs(track)])
    n = A.shape[0]          # 6
    h = np.zeros(L)
    h[0] = D[0, 0]
    Ak = np.eye(n)
    for k in range(1, L):
        h[k] = (C @ Ak @ B)[0, 0]
        Ak = A @ Ak
    T = np.zeros((L, L))
    for i in range(L):
        T[i, : i + 1] = h[i::-1]
    Phi = np.zeros((L, n))
    Ak = np.eye(n)
    for k in range(L):
        Phi[k] = (C @ Ak)[0]
        Ak = A @ Ak
    A_L = Ak
    zmT = np.zeros((L, n))
    Ak = np.eye(n)
    for t in range(L - 1, -1, -1):       # A^(L-1-t) B
        zmT[t] = (Ak @ B)[:, 0]
        Ak = A @ Ak
    zc = np.zeros((n * JMAX, L))
    lam = max(abs(np.linalg.eigvals(A_L)))
    J = int(np.clip(np.ceil(np.log(1e-4) / np.log(max(lam, 1e-12))), 2, JMAX))
    Ai = np.eye(n)                       # A_L^(j-1) for j = 1..J
    for j in range(1, J + 1):
        G = Phi @ Ai                     # [L, n]
        ip = JMAX - j                    # stack row group for shift j
        for c in range(n):
            zc[n * ip + c] = G[:, c]
        Ai = A_L @ Ai
    return T, zmT, zc

# ---------------------------------------------------------------- reverb host math
def _epoch_matrix_comb(fb, L=128):
    Lm = np.zeros((L, L))
    for q in range(L):
        y = np.zeros(L)
        prev = 0.0
        for p_ in range(L):
            y[p_] = (1.0 if p_ == q else 0.0) + fb * prev
            prev = y[p_]
        Lm[:, q] = y
    return Lm

def _epoch_matrix_ap(fb, L=128, quirk=False):
    Lm = np.zeros((L, L))
    for q in range(L):
        X = np.zeros(L)
        X[q] = 1.0
        y = np.zeros(L)
        yprev = 0.0
        xprev = 0.0
        for p_ in range(L):
            y[p_] = 0.0 if (quirk and p_ == 0) else (-fb * X[p_] + xprev + fb * yprev)
            yprev = y[p_]
            xprev = X[p_]
        Lm[:, q] = y
    return Lm

def _rev_tiles(d):
    M = -(-NP // d)
    T = -(-M // VEP)
    return T, T * VEP * d   # tile count, flat coverage

_COMB_COVER = max(_rev_tiles(d)[1] for d in COMB_DELAYS)
_AP0_COVER = _rev_tiles(AP_DELAYS[0])[1]
_AP1_COVER = _rev_tiles(AP_DELAYS[1])[1]
# ap81 reads apdram up to its own tile-grid coverage; ap240 only writes its
# grid's coverage -> size apdram to the max and zero the gap
_AP0_SIZE = max(_AP0_COVER, _AP1_COVER)


# ============================================================== device program
def build_program(with_collective=True, phase_limit=4, debug_taps=False,
                  no_accum=False):
    nc = bacc.Bacc("TRN2", target_bir_lowering=False, debug=False)
    dt = F32
    ao = mybir.AluOpType
    AF = mybir.ActivationFunctionType

    x = nc.declare_dram_parameter("x", [NP], dt, isOutput=False)
    thT = nc.declare_dram_parameter("thT", [128, 128], dt, isOutput=False)
    zm = nc.declare_dram_parameter("zm", [128, TAILD], dt, isOutput=False)
    zc = nc.declare_dram_parameter("zc", [TAILD * JMAX, 128], dt, isOutput=False)
    identp = nc.declare_dram_parameter("ident", [128, 128], dt, isOutput=False)
    lcT = nc.declare_dram_parameter("lcT", [128, 128], dt, isOutput=False)
    laT = nc.declare_dram_parameter("laT", [128, 128], dt, isOutput=False)
    laqT = nc.declare_dram_parameter("laqT", [128, 128], dt, isOutput=False)
    relpow = nc.declare_dram_parameter("relpow", [128, FC], dt, isOutput=False)
    wdry = nc.declare_dram_parameter("wdry", [128, 1], dt, isOutput=False)
    wwet = nc.declare_dram_parameter("wwet", [128, 1], dt, isOutput=False)
    out = nc.declare_dram_parameter("out", [CH], dt, isOutput=True)

    ydram = nc.dram_tensor("ydram", [_COMB_COVER], BF16)
    # f32: DMA-accumulate in bf16 faults (even for 4B-aligned rows); f32
    # accumulation is the known-good path
    wetdram = nc.dram_tensor("wetdram", [_COMB_COVER], dt)
    apdram = nc.dram_tensor("apdram", [_AP0_SIZE], BF16)
    wet2dram = nc.dram_tensor("wet2dram", [_AP1_COVER], BF16)
    mixdram = nc.dram_tensor("mixdram", [NP], BF16)
    zdram = nc.dram_tensor("zdram", [TAILD, 32 + F], BF16)
    rsdram = nc.dram_tensor("rsdram", [CH], BF16)
    if debug_taps:
        dbg_yeq = nc.dram_tensor("dbg_yeq", [NP], BF16)
        dbg_env = nc.dram_tensor("dbg_env", [NP], BF16)

    with tile.TileContext(nc) as tc, ExitStack() as ctx:
        cons = ctx.enter_context(tc.tile_pool(name="cons", bufs=1))
        bbf = ctx.enter_context(tc.tile_pool(name="bbf", bufs=4))
        ps = ctx.enter_context(tc.tile_pool(name="ps", bufs=4, space="PSUM"))
        tiny = ctx.enter_context(tc.tile_pool(name="tiny", bufs=2))

        # ---- constants to SBUF (f32 loads + one-time bf16 converts)
        t_thT = cons.tile([128, 128], dt, tag="thT")
        t_zm = cons.tile([128, TAILD], dt, tag="zm")
        t_zc = cons.tile([TAILD * JMAX, 128], dt, tag="zc")
        t_id = cons.tile([128, 128], dt, tag="ident")
        t_lcT = cons.tile([128, 128], dt, tag="lcT")
        t_laT = cons.tile([128, 128], dt, tag="laT")
        t_laqT = cons.tile([128, 128], dt, tag="laqT")
        t_relpow = cons.tile([128, FC], dt, tag="relpow")
        t_wdry = cons.tile([128, 1], dt, tag="wdry")
        t_wwet = cons.tile([128, 1], dt, tag="wwet")
        t_ones = cons.tile([1, 1], dt, tag="ones")
        t_zcol = cons.tile([128, 1], dt, tag="zcol")
        for t_, src in ((t_thT, thT), (t_zm, zm), (t_zc, zc), (t_id, identp),
                        (t_lcT, lcT), (t_laT, laT), (t_laqT, laqT),
                        (t_relpow, relpow), (t_wdry, wdry), (t_wwet, wwet)):
            nc.sync.dma_start(t_[:], src[:])
        nc.gpsimd.memset(t_ones[:], 1.0)
        nc.gpsimd.memset(t_zcol[:], 0.0)
        b_thT = cons.tile([128, 128], BF16, tag="thTb")
        b_zm = cons.tile([128, TAILD], BF16, tag="zmb")
        b_zc = cons.tile([TAILD * JMAX, 128], BF16, tag="zcb")
        b_lcT = cons.tile([128, 128], BF16, tag="lcTb")
        b_laT = cons.tile([128, 128], BF16, tag="laTb")
        b_laqT = cons.tile([128, 128], BF16, tag="laqTb")
        for bt, ft in ((b_thT, t_thT), (b_zm, t_zm), (b_zc, t_zc),
                       (b_lcT, t_lcT), (b_laT, t_laT), (b_laqT, t_laqT)):
            nc.scalar.copy(bt[:], ft[:])

        # ================= Phase A: load x -> stage -> PE transpose -> xL1
        # xL1[tau, c] = x[c*128 + tau] (bf16)
        xL1 = bbf.tile([128, F], BF16, tag="bigbf")
        x4 = x[:].rearrange("(w a b) -> w a b", a=128, b=128)  # [88,128,128]
        # gpsimd (Pool) cannot access PSUM -> psum copies only on Act/DVE
        cpeng = [nc.scalar.copy, nc.vector.tensor_copy]
        with tc.tile_pool(name="stg", bufs=4) as stg:
            for wq8 in range(11):
                s = stg.tile([128, 8, 128], dt, tag="ustg")
                nc.sync.dma_start(
                    s[:], x4[8 * wq8: 8 * wq8 + 8].rearrange("w a b -> a w b"))
                for half in range(2):
                    wq = 2 * wq8 + half
                    pt = ps.tile([128, 512], dt, tag="pstrans")
                    for wl in range(4):
                        nc.tensor.transpose(
                            pt[:, 128 * wl: 128 * wl + 128],
                            s[:, 4 * half + wl, :], t_id[:])
                    cpeng[wq % 2](xL1[:, 512 * wq: 512 * wq + 512], pt[:])

        # ================= Phase B: EQ matmuls (bf16), permuted output
        # ytr column 128w + b holds block 88b + w so that Phase C's contiguous
        # transposes emit lane-major data directly.
        with tc.tile_pool(name="pytr", bufs=1) as pytr:
            ytr = pytr.tile([128, F], dt, tag="bigf32")
            y0 = bbf.tile([128, F], BF16, tag="bigbf")
            SUBS = [440] * 6 + [176]   # 2816, all multiples of 88
            ytrV = ytr[:].rearrange("p (r c) -> p r c", c=128)  # [p, 88, 128]
            with tc.tile_pool(name="stk", bufs=2) as stkp:
                # zero left-pad of the z history once
                zpad = stkp.tile([TAILD, 32], BF16, tag="zpad")
                nc.gpsimd.memset(zpad[:], 0.0)
                nc.sync.dma_start(zdram[:, 0:32], zpad[:])
                for k in range(4):
                    base = FCB * k
                    zsb = stkp.tile([TAILD, FCB], BF16, tag="zsb")
                    off = 0
                    for sub in SUBS:
                        p1 = ps.tile([128, 512], dt, tag="psmm")
                        nc.tensor.matmul(p1[:, :sub], b_thT[:],
                                         xL1[:, base + off: base + off + sub])
                        nc.scalar.copy(y0[:, base + off: base + off + sub],
                                       p1[:, :sub])
                        # block end-states z = M @ x_block (exact, O(1) coeffs)
                        pz = ps.tile([128, 512], dt, tag="psmm")
                        nc.tensor.matmul(pz[0:TAILD, :sub], b_zm[:],
                                         xL1[:, base + off: base + off + sub])
                        nc.scalar.copy(zsb[:, off: off + sub], pz[0:TAILD, :sub])
                        off += sub
                    # this chunk's z rows to DRAM (sliding windows read them back)
                    nc.sync.dma_start(zdram[:, 32 + base: 32 + base + FCB], zsb[:])
                    sz = stkp.tile([TAILD * JMAX, FCB], BF16, tag="stack")
                    # single sliding-window DMA: stack row (i', c) = z row c
                    # shifted by (JMAX - i') blocks (lhsT zc rows match)
                    soff = 32 + base - JMAX
                    sap = [[1, JMAX], [32 + F, TAILD], [1, FCB]]
                    nc.sync.dma_start(
                        sz[:], bass.AP(tensor=zdram, offset=soff, ap=sap))
                    off = 0
                    for sub in SUBS:
                        p2 = ps.tile([128, 512], dt, tag="psmm")
                        nc.tensor.matmul(p2[:, :sub], b_zc[:],
                                         sz[:, off: off + sub])
                        # permuted write: psum col 88j+r (block base+off+88j+r)
                        # -> ytr col 128r + (base+off)/88 + j
                        s88 = sub // 88
                        Q0 = (base + off) // 88
                        nc.vector.tensor_tensor(
                            ytrV[:, :, Q0: Q0 + s88].rearrange("p r j -> p j r"),
                            y0[:, base + off: base + off + sub].rearrange(
                                "p (j r) -> p j r", r=88),
                            p2[:, :sub].rearrange("p (j r) -> p j r", r=88),
                            op=ao.add)
                        off += sub

            # ================= Phase C: contiguous transposes -> lane-major yeq
            yeq = bbf.tile([128, F], BF16, tag="bigbf")
            for wq in range(22):
                pt = ps.tile([128, 512], dt, tag="pstrans")
                for wl in range(4):
                    w = 4 * wq + wl
                    nc.tensor.transpose(pt[:, 128 * wl: 128 * wl + 128],
                                        ytr[:, 128 * w: 128 * w + 128], t_id[:])
                cpeng[wq % 2](yeq[:, 512 * wq: 512 * wq + 512], pt[:])
            if debug_taps:
                nc.sync.dma_start(
                    dbg_yeq[:].rearrange("(p f) -> p f", p=128), yeq[:])

        # ================= Phase D: compressor (env domain, bf16 tensors,
        # f32 scan coefficients; Gauss-Seidel chunk boundaries)
        lvl = bbf.tile([128, F], BF16, tag="bigbf")   # reuses xL1's slot
        for k8 in range(8):
            nc.scalar.activation(lvl[:, FC * k8: FC * (k8 + 1)],
                                 yeq[:, FC * k8: FC * (k8 + 1)], AF.Abs)
        env = bbf.tile([128, F], BF16, tag="bigbf")   # reuses y0's slot
        b_id = cons.tile([128, 128], BF16, tag="identb")
        nc.scalar.copy(b_id[:], t_id[:])
        # shifted identity: b_ids[k, m] = 1 iff m == k+1 (partition shift
        # via PE; avoids 2-byte-element partition-shift DMAs)
        b_ids = cons.tile([128, 128], BF16, tag="identsh")
        nc.gpsimd.memset(b_ids[:], 0.0)
        nc.scalar.copy(b_ids[:, 1:128], b_id[:, 0:127])
        with tc.tile_pool(name="relcp", bufs=1) as relcp, \
             tc.tile_pool(name="chk", bufs=2) as chk:
            relc = relcp.tile([128, FC], dt, tag="relc")
            nc.gpsimd.memset(relc[:], REL)
            rowsc = tiny.tile([1, 130], dt, tag="rowsc")
            irow = tiny.tile([1, 128], dt, tag="irow")
            nc.gpsimd.memset(rowsc[:], 0.0)
            relFrow = tiny.tile([1, 128], dt, tag="relF")
            nc.gpsimd.memset(relFrow[:], float(REL ** F))
            icol = tiny.tile([128, 1], dt, tag="icol")
            lanecol = tiny.tile([128, 1], BF16, tag="lanecol")

            # it0: all-release, zero lane inits; exact chain fix after
            for k in range(8):
                d1 = chk.tile([128, FC], BF16, tag="mtile")
                nc.scalar.activation(d1[:], lvl[:, FC * k: FC * (k + 1)],
                                     AF.Copy, bias=0.0, scale=float(1.0 - REL))
                init = 0.0 if k == 0 else env[:, FC * k - 1: FC * k]
                nc.vector.tensor_tensor_scan(
                    env[:, FC * k: FC * (k + 1)], relc[:], d1[:], init,
                    op0=ao.mult, op1=ao.add)
            pr = ps.tile([128, 512], dt, tag="psmm")
            nc.tensor.matmul(pr[:1, :128], env[:, F - 1: F], b_id[:])
            nc.scalar.copy(rowsc[0:1, 1:129], pr[:1, :128])
            nc.vector.tensor_tensor_scan(
                irow[:], relFrow[:], rowsc[0:1, 0:128], 0.0,
                op0=ao.mult, op1=ao.add)
            pc = ps.tile([128, 512], dt, tag="psmm")
            nc.tensor.matmul(pc[:128, :1], irow[:], t_ones[:])
            nc.scalar.copy(icol[:], pc[:128, :1])
            for k in range(8):
                isc = tiny.tile([128, 1], dt, tag="isc")
                nc.vector.tensor_scalar_mul(isc[:], icol[:],
                                            float(REL ** (FC * k)))
                nc.vector.scalar_tensor_tensor(
                    env[:, FC * k: FC * (k + 1)], t_relpow[:], isc[:, 0:1],
                    env[:, FC * k: FC * (k + 1)], op0=ao.mult, op1=ao.add)

            # policy iterations, pipelined: all masks + coefficient tensors
            # come from the previous iterate (Jacobi boundaries — same
            # convergence, validated); the 8 scans then chain serially with
            # current inits.  Act (w1/cf-even) overlaps DVE (is_gt/mul/scan);
            # odd-chunk cf on DVE to balance engine load.
            with tc.tile_pool(name="mp", bufs=8) as mp, \
                 tc.tile_pool(name="cfp", bufs=4) as cfp:
                for it in range(N_ITER):
                    psh = ps.tile([128, 512], dt, tag="psmm")
                    nc.tensor.matmul(psh[:, 0:1], b_ids[:], env[:, F - 1: F])
                    nc.scalar.copy(lanecol[:], psh[:, 0:1])
                    ms, cfs = [], []
                    for k in range(8):
                        base = FC * k
                        m = mp.tile([128, FC], BF16, tag="m")
                        bc = (lanecol[:, 0:1] if k == 0
                              else env[:, base - 1: base])
                        nc.vector.tensor_tensor(
                            m[:, 1:], lvl[:, base + 1: base + FC],
                            env[:, base: base + FC - 1], op=ao.is_gt)
                        nc.vector.tensor_tensor(
                            m[:, 0:1], lvl[:, base: base + 1], bc,
                            op=ao.is_gt)
                        ms.append(m)
                    for k in range(8):
                        base = FC * k
                        m = ms[k]
                        cf = cfp.tile([128, FC], dt, tag="cf")
                        if k % 2 == 0:
                            nc.scalar.activation(cf[:], m[:], AF.Copy,
                                                 bias=float(REL),
                                                 scale=float(ATK - REL))
                        else:
                            nc.vector.tensor_scalar(cf[:], m[:],
                                                    float(ATK - REL),
                                                    float(REL),
                                                    op0=ao.mult, op1=ao.add)
                        cfs.append(cf)
                        # w1 then ct, in place on the mask tile
                        nc.scalar.activation(
                            m[:], m[:], AF.Copy, bias=float(1.0 - REL),
                            scale=float((1.0 - ATK) - (1.0 - REL)))
                        nc.vector.tensor_mul(m[:], m[:],
                                             lvl[:, base: base + FC])
                        # scan for chunk k-1 interleaves with chunk k coeffs
                        if k >= 1:
                            b2 = FC * (k - 1)
                            bcs = (lanecol[:, 0:1] if k == 1
                                   else env[:, b2 - 1: b2])
                            nc.vector.tensor_tensor_scan(
                                env[:, b2: b2 + FC], cfs[k - 1][:],
                                ms[k - 1][:], bcs, op0=ao.mult, op1=ao.add)
                    nc.vector.tensor_tensor_scan(
                        env[:, FC * 7: F], cfs[7][:], ms[7][:],
                        env[:, FC * 7 - 1: FC * 7], op0=ao.mult, op1=ao.add)

            if debug_taps:
                nc.sync.dma_start(
                    dbg_env[:].rearrange("(p f) -> p f", p=128), env[:])
            # gr and y = x * gr (in-place onto yeq); g in bf16 (ln sees
            # values >= 1, so 0.4% input rounding is harmless)
            for k in range(8):
                base = FC * k
                g = chk.tile([128, FC], BF16, tag="mtile")
                nc.vector.tensor_scalar(g[:], env[:, base: base + FC],
                                        float(THR), float(1.0 / THR),
                                        op0=ao.max, op1=ao.mult)
                nc.scalar.activation(g[:], g[:], AF.Ln)
                nc.scalar.activation(g[:], g[:], AF.Exp,
                                     bias=0.0, scale=float(GR_EXP))
                nc.vector.tensor_mul(yeq[:, base: base + FC],
                                     yeq[:, base: base + FC], g[:])
        ycomp = yeq

        # ================= Phase E: reverb (bf16; blended by wwet)
        with tc.tile_pool(name="rvin", bufs=4) as rvin, \
             tc.tile_pool(name="rvout", bufs=3) as rvout:
            ydv = ydram[0:NP].rearrange("(p f) -> p f", p=128)
            for k8 in range(8):
                nc.sync.dma_start(ydv[:, FC * k8: FC * (k8 + 1)],
                                  ycomp[:, FC * k8: FC * (k8 + 1)])
            zt = rvin.tile([128, 1, 2030], BF16, tag="rv_in")
            nc.gpsimd.memset(zt[:], 0.0)
            tail = _COMB_COVER - NP
            tf = tail // 2030
            nc.sync.dma_start(
                ydram[NP: NP + tf * 2030].rearrange("(o f) -> o f", o=tf),
                zt[0:tf, 0, :])
            rem = tail - tf * 2030
            if rem:
                nc.sync.dma_start(
                    ydram[NP + tf * 2030:].rearrange("(o f) -> o f", o=1),
                    zt[tf: tf + 1, 0, 0:rem])

            def epoch_filter(src_dram, dst_dram, d, lhsT_t0, lhsT, accum,
                             G=1, in_dt=BF16, out_dt=BF16):
                Tt, cover = _rev_tiles(d)
                packmm = max(1, 512 // d)   # tiles per matmul (pack*d <= 512)

                def do_group(t, g):
                    # tiles t..t+g-1 (t>=1: warmup rows; t==0 solo, no warmup)
                    it_ = rvin.tile([128, g, d], in_dt, tag="rv_in")
                    if t == 0:
                        nc.sync.dma_start(
                            it_[:, 0, :],
                            src_dram[0: 128 * d].rearrange("(e i) -> e i",
                                                           e=128))
                    else:
                        soff = (VEP * t - WEP) * d
                        nc.sync.dma_start(
                            it_[:], bass.AP(tensor=src_dram, offset=soff,
                                            ap=[[d, 128], [VEP * d, g],
                                                [1, d]]))
                    ot = rvout.tile([128, g, d], out_dt, tag="rv_out")
                    lt = lhsT_t0 if t == 0 else lhsT
                    j = 0
                    while j < g:
                        pk = min(packmm, g - j)
                        sub = pk * d
                        offd = 0
                        while offd < d * pk:   # chunk if d > 512
                            s2 = min(512, sub - offd)
                            pe = ps.tile([128, 512], dt, tag="psmm")
                            rhs = (it_[:, j, offd: offd + s2] if pk == 1
                                   else it_[:, j: j + pk, :])
                            o2 = (ot[:, j, offd: offd + s2] if pk == 1
                                  else ot[:, j: j + pk, :])
                            ev = cpeng[(t + j) % 2]
                            if pk == 1:
                                nc.tensor.matmul(pe[:, :s2], lt[:], rhs)
                                ev(o2, pe[:, :s2])
                            else:
                                nc.tensor.matmul(
                                    pe[:, :sub].rearrange(
                                        "p (j i) -> p j i", j=pk), lt[:], rhs)
                                ev(o2, pe[:, :sub].rearrange(
                                    "p (j i) -> p j i", j=pk))
                            offd += s2
                        j += pk
                    if t == 0:
                        rows = ot[0:VEP, 0, :]
                        dst = dst_dram[0: VEP * d].rearrange("(e i) -> e i",
                                                             e=VEP)
                        eng = nc.gpsimd if accum else nc.sync
                        eng.dma_start(dst, rows,
                                      **({"accum_op": ao.add} if accum else {}))
                    else:
                        if not accum:
                            dap = bass.AP(tensor=dst_dram, offset=VEP * t * d,
                                          ap=[[d, VEP], [VEP * d, g], [1, d]])
                            nc.sync.dma_start(dap, ot[WEP:128, :, :])
                        else:
                            # SWDGE ring holds ~1024 descriptors; keep each
                            # accumulating DMA under ~700 (6*116 descs)
                            j0 = 0
                            while j0 < g:
                                gg = min(6, g - j0)
                                dap = bass.AP(
                                    tensor=dst_dram,
                                    offset=(VEP * (t + j0)) * d,
                                    ap=[[d, VEP], [VEP * d, gg], [1, d]])
                                nc.gpsimd.dma_start(
                                    dap, ot[WEP:128, j0: j0 + gg, :],
                                    accum_op=ao.add)
                                j0 += gg

                do_group(0, 1)
                t = 1
                while t < Tt:
                    g = min(G, Tt - t)
                    do_group(t, g)
                    t += g

            combs = sorted(COMB_DELAYS, key=lambda d: -_rev_tiles(d)[1])
            for ci, d in enumerate(combs):
                epoch_filter(ydram, wetdram, d, b_lcT, b_lcT,
                             accum=(ci > 0 and not no_accum), out_dt=dt)
            epoch_filter(wetdram, apdram, AP_DELAYS[0], t_laqT, t_laT, False,
                         G=8, in_dt=dt)
            if _AP0_SIZE > _AP0_COVER:
                gap = _AP0_SIZE - _AP0_COVER
                ztg = rvin.tile([128, 1, 2030], BF16, tag="rv_in")
                nc.gpsimd.memset(ztg[:], 0.0)
                nc.sync.dma_start(
                    apdram[_AP0_COVER:].rearrange("(o f) -> o f", o=1),
                    ztg[0:1, 0, 0:gap])
            epoch_filter(apdram, wet2dram, AP_DELAYS[1], b_laqT, b_laT, False,
                         G=24)

        # ============= Phase F: mix + collective + clip (chunked to
        # pipeline wet read / blend / mix write)
        with tc.tile_pool(name="finp", bufs=2) as finp:
            wetl2 = bbf.tile([128, F], BF16, tag="bigbf")
            nc.scalar.activation(ycomp[:], ycomp[:], AF.Copy,
                                 scale=t_wdry[:, 0:1])
            w2v = wet2dram[0:NP].rearrange("(p f) -> p f", p=128)
            mdv = mixdram[0:NP].rearrange("(p f) -> p f", p=128)
            for k in range(4):
                c0, c1 = FCB * k, FCB * (k + 1)
                nc.sync.dma_start(wetl2[:, c0:c1], w2v[:, c0:c1])
                nc.vector.scalar_tensor_tensor(
                    wetl2[:, c0:c1], wetl2[:, c0:c1], t_wwet[:, 0:1],
                    ycomp[:, c0:c1], op0=ao.mult, op1=ao.add)
                nc.sync.dma_start(mdv[:, c0:c1], wetl2[:, c0:c1])
            if with_collective:
                nc.gpsimd.collective_compute(
                    "ReduceScatter", ao.add,
                    replica_groups=[[0, 2, 4, 6], [1, 3, 5, 7]],
                    ins=[mixdram[0:NP].opt()],
                    outs=[rsdram.ap().opt()],
                )
            else:
                nc.sync.dma_start(rsdram[:], mixdram[0:CH])
            rs2 = rsdram[:].rearrange("(p f) -> p f", p=128)   # [128, 2816]
            o2 = out[:].rearrange("(p f) -> p f", p=128)
            for hh in range(2):
                oc = finp.tile([128, FC], BF16, tag="f_oc")
                ocf = finp.tile([128, FC], dt, tag="f_ocf")
                nc.sync.dma_start(oc[:], rs2[:, FC * hh: FC * (hh + 1)])
                nc.vector.tensor_scalar(ocf[:], oc[:], float(-CEIL), float(CEIL),
                                        op0=ao.max, op1=ao.min)
                nc.sync.dma_start(o2[:, FC * hh: FC * (hh + 1)], ocf[:])

    nc.compile()
    return nc


# ============================================================== host wrapper
_CACHE = {}

def _get_program():
    if "nc" not in _CACHE:
        _CACHE["nc"] = build_program()
    return _CACHE["nc"]


def _host_consts():
    if "consts" in _CACHE:
        return _CACHE["consts"]
    ident = np.eye(128, dtype=np.float32)
    Lc = np.ascontiguousarray((0.25 * _epoch_matrix_comb(FB)).T.astype(np.float32))
    La = np.ascontiguousarray(_epoch_matrix_ap(FB).T.astype(np.float32))
    Laq = np.ascontiguousarray(_epoch_matrix_ap(FB, quirk=True).T.astype(np.float32))
    relpow = np.ascontiguousarray(np.broadcast_to(
        (REL ** (np.arange(FC, dtype=np.float64) + 1.0)).astype(np.float32),
        (128, FC)))
    eqc = {}
    for t in range(4):
        T, zmT_, zc_ = _track_eq_consts(t)
        eqc[t] = (np.ascontiguousarray(T.T.astype(np.float32)),
                  np.ascontiguousarray(zmT_.astype(np.float32)),
                  np.ascontiguousarray(zc_.astype(np.float32)))
    _CACHE["consts"] = (ident, Lc, La, Laq, relpow, eqc)
    return _CACHE["consts"]


def kernel(tracks, volumes, pans):
    tracks = np.ascontiguousarray(np.asarray(tracks, np.float32))
    volumes = np.asarray(volumes, np.float32)
    pans = np.asarray(pans, np.float32)

    angle = (pans.astype(np.float64) + 1.0) * 0.25 * math.pi
    lg, rg = np.cos(angle), np.sin(angle)
    ident, Lc, La, Laq, relpow, eqc = _host_consts()

    in_maps = []
    for core in range(8):
        t, ch = core // 2, core % 2
        xpad = np.zeros(NP, np.float32)
        xpad[:N] = tracks[t, ch]
        thT_np, zm_np, zc_np = eqc[t]
        w = float(volumes[t]) * float(lg[t] if ch == 0 else rg[t])
        has_rev = t >= 2
        w_dry = w * (1.0 - WET) if has_rev else w
        w_wet = w * WET if has_rev else 0.0
        in_maps.append({
            "x": xpad, "thT": thT_np, "zm": zm_np, "zc": zc_np,
            "ident": ident, "lcT": Lc, "laT": La, "laqT": Laq,
            "relpow": relpow,
            "wdry": np.full((128, 1), w_dry, np.float32),
            "wwet": np.full((128, 1), w_wet, np.float32),
        })

    nc = _get_program()
    res = run_bass_kernel_spmd(nc, in_maps, list(range(8)))

    outp = np.zeros((2, N), np.float32)
    for ch in range(2):
        full = np.concatenate([res.results[2 * q + ch]["out"] for q in range(4)])
        outp[ch] = full[:N]
    return outp



# revision 10
# speedup vs baseline: 1.1254x; 1.1254x over previous
"""Trainium2 Bass kernel for nn_AudioMixer (4-track stereo mixer:
per-track 3-stage biquad EQ -> compressor -> Schroeder reverb on tracks 2,3
-> pan/volume mix -> limiter clip).

Sharding: core c = (track c//2, channel c%2) — each of the 8 cores processes
one full (track, channel) row of 1.44M samples end-to-end, then a
ReduceScatter over channel groups {0,2,4,6} / {1,3,5,7} sums the 4 weighted
tracks per channel; each core clips + writes a quarter of its channel.

Algorithms (validated against the jax reference):
 - EQ: combined per-track state-space cascade (order 6). Per 128-sample block:
   zero-state response via a lower-triangular Toeplitz matmul (bf16) on the
   PE; cross-block state corrections via two more matmuls whose rhs are
   stacked shifted "tail" rows. The correction add writes through a permuted
   AP view so that column c = 128w+b holds block 88b+w; the contiguous
   128x128 transposes of Phase C then directly produce the lane-major
   layout (lane p = samples [11264p, 11264(p+1))) with no DRAM round-trip.
 - Compressor: attack/release envelope via policy iteration in the
   u = env - lvl domain: u[n] = (d[n] - u[n-1]) * (-c[n]) with
   d[n] = lvl[n]-lvl[n-1], so each iteration is only mask -> affine -> scan
   (no per-iteration multiply). it0 (all-release) gets an exact cross-lane
   chain fix via REL-power decay; later iterations chain via current
   (Gauss-Seidel) boundary columns.
 - Reverb: comb y[n] = x[n] + fb*y[n-d] over "epochs" of d samples is a
   lower-triangular matmul over 128 stacked epochs (fb^(p-q)); tiles of 128
   epochs overlap by WEP warmup epochs. Same for both allpasses (the
   reference's first-epoch-zero quirk gets its own matrix for tile 0).
   All reverb DRAM scratch + matmuls in bf16; comb accumulation via
   bf16 accumulating DMAs.
"""
import math
from contextlib import ExitStack

import numpy as np

import concourse.bass as bass
import concourse.bacc as bacc
import concourse.mybir as mybir
import concourse.tile as tile
from concourse.bass_utils import run_bass_kernel_spmd

F32 = mybir.dt.float32
BF16 = mybir.dt.bfloat16

# ---------------------------------------------------------------- constants
SR = 48000
N = 1_440_000
NP = 128 * 11264          # padded row length (1441792)
F = 11264                 # per-lane length (128 lanes)
FCB = 2816                # EQ free-chunk (4 chunks); 2816 = 32*88
FC = 1408                 # compressor free-chunk (8 chunks)
CH = NP // 4              # ReduceScatter chunk per core = 360448

# reverb pair-split: core c (c<4, "partner") processes the FIRST half of the
# reverb row owned by core c+4 ("owner" keeps its second half).  The rin
# buffer per core = PAD warmup samples + its half (HS); uniform SPMD program,
# per-core behavior only via parameter values (masks / quirk matrices).
HSL = 64                  # half-signal lanes
HS = HSL * F              # 720896
PADL = 6                  # warmup lanes (67584 >= 24*2030 + ap tails)
PAD = PADL * F
RL = PADL + HSL           # 70 lanes
RN = RL * F               # 788480 per-core reverb length

ATK = math.exp(-1.0 / (10.0 * 0.001 * SR))
REL = math.exp(-1.0 / (100.0 * 0.001 * SR))
THR = 10.0 ** (-18.0 / 20.0)
GR_EXP = 1.0 / 4.0 - 1.0
_BASE = int(SR * 0.03)
COMB_DELAYS = [_BASE, int(_BASE * 1.13), int(_BASE * 1.27), int(_BASE * 1.41)]
AP_DELAYS = [int(SR * 0.005), int(SR * 0.0017)]
FB = 0.3 + 0.5 * 0.6
WET = 0.3
CEIL = 10.0 ** (-1.0 / 20.0)

N_ITER = 3                # compressor policy iterations
TAILD = 6                 # tail rows for EQ correction
JMAX = 12                 # correction shift terms (zero-padded per track)
VEP = 104               # valid epochs per reverb tile
WEP = 24                # warmup epochs per tile

# ---------------------------------------------------------------- EQ host math
def _peak_coefs(freq, gain_db, q):
    A = 10.0 ** (gain_db / 40.0)
    w0 = 2.0 * math.pi * freq / SR
    al = math.sin(w0) / (2.0 * q)
    a0 = 1.0 + al / A
    return ((1.0 + al * A) / a0, -2.0 * math.cos(w0) / a0, (1.0 - al * A) / a0,
            -2.0 * math.cos(w0) / a0, (1.0 - al / A) / a0)

_IDENT = (1.0, 0.0, 0.0, 0.0, 0.0)
_PRESETS = {
    0: [(300.0, -3.0, 0.7), (3000.0, 3.0, 1.0), (8000.0, 2.0, 0.7)],
    1: [(80.0, 2.0, 0.7), (5000.0, 1.0, 1.0)],
    2: [(200.0, -2.0, 0.7), (6000.0, -1.0, 0.7)],
    3: [(1000.0, 2.0, 1.0)],
}

def _stage_coefs(track):
    bands = [_peak_coefs(*b) for b in _PRESETS[track]]
    bands += [_IDENT] * (3 - len(bands))
    return bands

def _biquad_ss(c):
    b0, b1, b2, a1, a2 = [float(v) for v in c]
    A = np.array([[-a1, 1.0], [-a2, 0.0]])
    B = np.array([[b1 - a1 * b0], [b2 - a2 * b0]])
    C = np.array([[1.0, 0.0]])
    D = np.array([[b0]])
    return A, B, C, D

def _cascade(ss_list):
    A1, B1, C1, D1 = ss_list[0]
    for A2, B2, C2, D2 in ss_list[1:]:
        n1, n2 = A1.shape[0], A2.shape[0]
        A = np.zeros((n1 + n2, n1 + n2))
        A[:n1, :n1] = A1
        A[n1:, :n1] = B2 @ C1
        A[n1:, n1:] = A2
        B = np.vstack([B1, B2 @ D1])
        C = np.hstack([D2 @ C1, C2])
        D = D2 @ D1
        A1, B1, C1, D1 = A, B, C, D
    return A1, B1, C1, D1

def _track_eq_consts(track, L=128):
    """Returns (T, zmT, zc):
    T    [L, L]   lower-triangular Toeplitz of the impulse response
    zmT  [L, n]   lhsT of the end-state matmul: z_b = M @ x_block,
                  M[:, t] = A^(L-1-t) B  (zmT[t, c] = M[c, t])
    zc   [n*JMAX, L]  correction lhsT: row (i', c) pairs with a z-stack row
                  holding z shifted by (JMAX - i') blocks; the correction for
                  block b is Phi @ s_{b-1} with s_{b-1} ~ sum_j A_L^(j-1)
                  z_{b-j}.  All entries O(1) -> bf16-safe (unlike the old
                  tail-probe fit whose inverse had entries ~500)."""
    A, B, C, D = _cascade([_biquad_ss(c) for c in _stage_coefs(track)])
    n = A.shape[0]          # 6
    h = np.zeros(L)
    h[0] = D[0, 0]
    Ak = np.eye(n)
    for k in range(1, L):
        h[k] = (C @ Ak @ B)[0, 0]
        Ak = A @ Ak
    T = np.zeros((L, L))
    for i in range(L):
        T[i, : i + 1] = h[i::-1]
    Phi = np.zeros((L, n))
    Ak = np.eye(n)
    for k in range(L):
        Phi[k] = (C @ Ak)[0]
        Ak = A @ Ak
    A_L = Ak
    zmT = np.zeros((L, n))
    Ak = np.eye(n)
    for t in range(L - 1, -1, -1):       # A^(L-1-t) B
        zmT[t] = (Ak @ B)[:, 0]
        Ak = A @ Ak
    zc = np.zeros((n * JMAX, L))
    lam = max(abs(np.linalg.eigvals(A_L)))
    J = int(np.clip(np.ceil(np.log(1e-4) / np.log(max(lam, 1e-12))), 2, JMAX))
    Ai = np.eye(n)                       # A_L^(j-1) for j = 1..J
    for j in range(1, J + 1):
        G = Phi @ Ai                     # [L, n]
        ip = JMAX - j                    # stack row group for shift j
        for c in range(n):
            zc[n * ip + c] = G[:, c]
        Ai = A_L @ Ai
    return T, zmT, zc

# ---------------------------------------------------------------- reverb host math
def _epoch_matrix_comb(fb, L=128):
    Lm = np.zeros((L, L))
    for q in range(L):
        y = np.zeros(L)
        prev = 0.0
        for p_ in range(L):
            y[p_] = (1.0 if p_ == q else 0.0) + fb * prev
            prev = y[p_]
        Lm[:, q] = y
    return Lm

def _epoch_matrix_ap(fb, L=128, quirk=False):
    Lm = np.zeros((L, L))
    for q in range(L):
        X = np.zeros(L)
        X[q] = 1.0
        y = np.zeros(L)
        yprev = 0.0
        xprev = 0.0
        for p_ in range(L):
            y[p_] = 0.0 if (quirk and p_ == 0) else (-fb * X[p_] + xprev + fb * yprev)
            yprev = y[p_]
            xprev = X[p_]
        Lm[:, q] = y
    return Lm

def _rev_tiles(d):
    M = -(-RN // d)
    T = -(-M // VEP)
    return T, T * VEP * d   # tile count, flat coverage

_COMB_COVER = max(_rev_tiles(d)[1] for d in COMB_DELAYS)
_AP0_COVER = _rev_tiles(AP_DELAYS[0])[1]
_AP1_COVER = _rev_tiles(AP_DELAYS[1])[1]
# ap81 reads apdram up to its own tile-grid coverage; ap240 only writes its
# grid's coverage -> size apdram to the max and zero the gap
_AP0_SIZE = max(_AP0_COVER, _AP1_COVER)
# ydram zero tail, rounded up to a [128, x] tile write
_YTAIL = _COMB_COVER - RN
_YTAILC = -(-_YTAIL // 128)
_YDRAM_SIZE = RN + 128 * _YTAILC


# ============================================================== device program
def build_program(with_collective=True, phase_limit=4, debug_taps=False,
                  no_accum=False):
    nc = bacc.Bacc("TRN2", target_bir_lowering=False, debug=False)
    dt = F32
    ao = mybir.AluOpType
    AF = mybir.ActivationFunctionType

    x = nc.declare_dram_parameter("x", [NP], dt, isOutput=False)
    thT = nc.declare_dram_parameter("thT", [128, 128], dt, isOutput=False)
    zm = nc.declare_dram_parameter("zm", [128, TAILD], dt, isOutput=False)
    zc = nc.declare_dram_parameter("zc", [TAILD * JMAX, 128], dt, isOutput=False)
    identp = nc.declare_dram_parameter("ident", [128, 128], dt, isOutput=False)
    lcT = nc.declare_dram_parameter("lcT", [128, 128], dt, isOutput=False)
    laT = nc.declare_dram_parameter("laT", [128, 128], dt, isOutput=False)
    laqT = nc.declare_dram_parameter("laqT", [128, 128], dt, isOutput=False)
    laT2 = nc.declare_dram_parameter("laT2", [128, 128], dt, isOutput=False)
    laqT2 = nc.declare_dram_parameter("laqT2", [128, 128], dt, isOutput=False)
    relpow = nc.declare_dram_parameter("relpow", [128, FC], dt, isOutput=False)
    # per-core scalars: ln(wdry) folded into the compressor's Exp bias;
    # mrecv/mown select the rin source (partner: AG slot 1 / owner: local
    # shifted ycomp); placed puts the wet half into mix lanes 0-63 (partner)
    # or 64-127 (owner)
    lnwdry = nc.declare_dram_parameter("lnwdry", [128, 1], dt, isOutput=False)
    mrecv = nc.declare_dram_parameter("mrecv", [128, 1], dt, isOutput=False)
    mown = nc.declare_dram_parameter("mown", [128, 1], dt, isOutput=False)
    placed = nc.declare_dram_parameter("placed", [128, 1], dt, isOutput=False)
    out = nc.declare_dram_parameter("out", [CH], dt, isOutput=True)

    senddram = nc.dram_tensor("senddram", [RN], BF16)
    agdram = nc.dram_tensor("agdram", [2 * RN], BF16)
    ydram = nc.dram_tensor("ydram", [_YDRAM_SIZE], BF16)
    # f32: DMA-accumulate in bf16 faults (even for 4B-aligned rows); f32
    # accumulation is the known-good path
    wetdram = nc.dram_tensor("wetdram", [_COMB_COVER], dt)
    apdram = nc.dram_tensor("apdram", [_AP0_SIZE], BF16)
    wet2dram = nc.dram_tensor("wet2dram", [_AP1_COVER], BF16)
    mixdram = nc.dram_tensor("mixdram", [NP], BF16)
    zdram = nc.dram_tensor("zdram", [TAILD, 32 + F], BF16)
    rsdram = nc.dram_tensor("rsdram", [CH], BF16)
    if debug_taps:
        dbg_yeq = nc.dram_tensor("dbg_yeq", [NP], BF16)
        dbg_env = nc.dram_tensor("dbg_env", [NP], BF16)

    with tile.TileContext(nc) as tc, ExitStack() as ctx:
        cons = ctx.enter_context(tc.tile_pool(name="cons", bufs=1))
        bbf = ctx.enter_context(tc.tile_pool(name="bbf", bufs=4))
        ps = ctx.enter_context(tc.tile_pool(name="ps", bufs=4, space="PSUM"))
        tiny = ctx.enter_context(tc.tile_pool(name="tiny", bufs=2))

        # ---- constants to SBUF (f32 loads + one-time bf16 converts)
        t_thT = cons.tile([128, 128], dt, tag="thT")
        t_zm = cons.tile([128, TAILD], dt, tag="zm")
        t_zc = cons.tile([TAILD * JMAX, 128], dt, tag="zc")
        t_id = cons.tile([128, 128], dt, tag="ident")
        t_lcT = cons.tile([128, 128], dt, tag="lcT")
        t_laT = cons.tile([128, 128], dt, tag="laT")
        t_laqT = cons.tile([128, 128], dt, tag="laqT")
        t_laT2 = cons.tile([128, 128], dt, tag="laT2")
        t_laqT2 = cons.tile([128, 128], dt, tag="laqT2")
        t_relpow = cons.tile([128, FC], dt, tag="relpow")
        t_lnw = cons.tile([128, 1], dt, tag="lnwdry")
        t_mrecv = cons.tile([128, 1], dt, tag="mrecv")
        t_mown = cons.tile([128, 1], dt, tag="mown")
        t_placed = cons.tile([128, 1], dt, tag="placed")
        t_ones = cons.tile([1, 1], dt, tag="ones")
        t_zcol = cons.tile([128, 1], dt, tag="zcol")
        for t_, src in ((t_thT, thT), (t_zm, zm), (t_zc, zc), (t_id, identp),
                        (t_lcT, lcT), (t_laT, laT), (t_laqT, laqT),
                        (t_laT2, laT2), (t_laqT2, laqT2),
                        (t_relpow, relpow), (t_lnw, lnwdry),
                        (t_mrecv, mrecv), (t_mown, mown), (t_placed, placed)):
            nc.sync.dma_start(t_[:], src[:])
        nc.gpsimd.memset(t_ones[:], 1.0)
        nc.gpsimd.memset(t_zcol[:], 0.0)
        b_thT = cons.tile([128, 128], BF16, tag="thTb")
        b_zm = cons.tile([128, TAILD], BF16, tag="zmb")
        b_zc = cons.tile([TAILD * JMAX, 128], BF16, tag="zcb")
        b_lcT = cons.tile([128, 128], BF16, tag="lcTb")
        b_laT2 = cons.tile([128, 128], BF16, tag="laT2b")
        b_laqT2 = cons.tile([128, 128], BF16, tag="laqT2b")
        for bt, ft in ((b_thT, t_thT), (b_zm, t_zm), (b_zc, t_zc),
                       (b_lcT, t_lcT), (b_laT2, t_laT2), (b_laqT2, t_laqT2)):
            nc.scalar.copy(bt[:], ft[:])

        # ================= Phase A: load x -> stage -> PE transpose -> xL1
        # xL1[tau, c] = x[c*128 + tau] (bf16)
        xL1 = bbf.tile([128, F], BF16, tag="bigbf")
        x4 = x[:].rearrange("(w a b) -> w a b", a=128, b=128)  # [88,128,128]
        # gpsimd (Pool) cannot access PSUM -> psum copies only on Act/DVE
        cpeng = [nc.scalar.copy, nc.vector.tensor_copy]
        with tc.tile_pool(name="stg", bufs=4) as stg:
            for wq8 in range(11):
                s = stg.tile([128, 8, 128], dt, tag="ustg")
                nc.sync.dma_start(
                    s[:], x4[8 * wq8: 8 * wq8 + 8].rearrange("w a b -> a w b"))
                for half in range(2):
                    wq = 2 * wq8 + half
                    pt = ps.tile([128, 512], dt, tag="pstrans")
                    for wl in range(4):
                        nc.tensor.transpose(
                            pt[:, 128 * wl: 128 * wl + 128],
                            s[:, 4 * half + wl, :], t_id[:])
                    cpeng[wq % 2](xL1[:, 512 * wq: 512 * wq + 512], pt[:])

        # ================= Phase B: EQ matmuls (bf16), permuted output
        # ytr column 128w + b holds block 88b + w so that Phase C's contiguous
        # transposes emit lane-major data directly.
        with tc.tile_pool(name="pytr", bufs=1) as pytr:
            ytr = pytr.tile([128, F], dt, tag="bigf32")
            y0 = bbf.tile([128, F], BF16, tag="bigbf")
            SUBS = [440] * 6 + [176]   # 2816, all multiples of 88
            ytrV = ytr[:].rearrange("p (r c) -> p r c", c=128)  # [p, 88, 128]
            with tc.tile_pool(name="stk", bufs=2) as stkp:
                # zero left-pad of the z history once
                zpad = stkp.tile([TAILD, 32], BF16, tag="zpad")
                nc.gpsimd.memset(zpad[:], 0.0)
                nc.sync.dma_start(zdram[:, 0:32], zpad[:])
                for k in range(4):
                    base = FCB * k
                    zsb = stkp.tile([TAILD, FCB], BF16, tag="zsb")
                    off = 0
                    for sub in SUBS:
                        p1 = ps.tile([128, 512], dt, tag="psmm")
                        nc.tensor.matmul(p1[:, :sub], b_thT[:],
                                         xL1[:, base + off: base + off + sub])
                        nc.scalar.copy(y0[:, base + off: base + off + sub],
                                       p1[:, :sub])
                        # block end-states z = M @ x_block (exact, O(1) coeffs)
                        pz = ps.tile([128, 512], dt, tag="psmm")
                        nc.tensor.matmul(pz[0:TAILD, :sub], b_zm[:],
                                         xL1[:, base + off: base + off + sub])
                        nc.scalar.copy(zsb[:, off: off + sub], pz[0:TAILD, :sub])
                        off += sub
                    # this chunk's z rows to DRAM (sliding windows read them back)
                    nc.sync.dma_start(zdram[:, 32 + base: 32 + base + FCB], zsb[:])
                    sz = stkp.tile([TAILD * JMAX, FCB], BF16, tag="stack")
                    # single sliding-window DMA: stack row (i', c) = z row c
                    # shifted by (JMAX - i') blocks (lhsT zc rows match)
                    soff = 32 + base - JMAX
                    sap = [[1, JMAX], [32 + F, TAILD], [1, FCB]]
                    nc.sync.dma_start(
                        sz[:], bass.AP(tensor=zdram, offset=soff, ap=sap))
                    off = 0
                    for sub in SUBS:
                        p2 = ps.tile([128, 512], dt, tag="psmm")
                        nc.tensor.matmul(p2[:, :sub], b_zc[:],
                                         sz[:, off: off + sub])
                        # permuted write: psum col 88j+r (block base+off+88j+r)
                        # -> ytr col 128r + (base+off)/88 + j
                        s88 = sub // 88
                        Q0 = (base + off) // 88
                        nc.vector.tensor_tensor(
                            ytrV[:, :, Q0: Q0 + s88].rearrange("p r j -> p j r"),
                            y0[:, base + off: base + off + sub].rearrange(
                                "p (j r) -> p j r", r=88),
                            p2[:, :sub].rearrange("p (j r) -> p j r", r=88),
                            op=ao.add)
                        off += sub

            # ================= Phase C: contiguous transposes -> lane-major yeq
            yeq = bbf.tile([128, F], BF16, tag="bigbf")
            for wq in range(22):
                pt = ps.tile([128, 512], dt, tag="pstrans")
                for wl in range(4):
                    w = 4 * wq + wl
                    nc.tensor.transpose(pt[:, 128 * wl: 128 * wl + 128],
                                        ytr[:, 128 * w: 128 * w + 128], t_id[:])
                cpeng[wq % 2](yeq[:, 512 * wq: 512 * wq + 512], pt[:])
            if debug_taps:
                nc.sync.dma_start(
                    dbg_yeq[:].rearrange("(p f) -> p f", p=128), yeq[:])

        # ================= Phase D: compressor (env domain, bf16 tensors,
        # f32 scan coefficients; Gauss-Seidel chunk boundaries)
        lvl = bbf.tile([128, F], BF16, tag="bigbf")   # reuses xL1's slot
        for k8 in range(8):
            nc.scalar.activation(lvl[:, FC * k8: FC * (k8 + 1)],
                                 yeq[:, FC * k8: FC * (k8 + 1)], AF.Abs)
        env = bbf.tile([128, F], BF16, tag="bigbf")   # reuses y0's slot
        b_id = cons.tile([128, 128], BF16, tag="identb")
        nc.scalar.copy(b_id[:], t_id[:])
        # shifted identity: b_ids[k, m] = 1 iff m == k+1 (partition shift
        # via PE; avoids 2-byte-element partition-shift DMAs)
        b_ids = cons.tile([128, 128], BF16, tag="identsh")
        nc.gpsimd.memset(b_ids[:], 0.0)
        nc.scalar.copy(b_ids[:, 1:128], b_id[:, 0:127])
        with tc.tile_pool(name="relcp", bufs=1) as relcp, \
             tc.tile_pool(name="chk", bufs=2) as chk:
            relc = relcp.tile([128, FC], dt, tag="relc")
            nc.gpsimd.memset(relc[:], REL)
            rowsc = tiny.tile([1, 130], dt, tag="rowsc")
            irow = tiny.tile([1, 128], dt, tag="irow")
            nc.gpsimd.memset(rowsc[:], 0.0)
            relFrow = tiny.tile([1, 128], dt, tag="relF")
            nc.gpsimd.memset(relFrow[:], float(REL ** F))
            icol = tiny.tile([128, 1], dt, tag="icol")
            lanecol = tiny.tile([128, 1], BF16, tag="lanecol")

            # it0: all-release, zero lane inits; exact chain fix after
            for k in range(8):
                d1 = chk.tile([128, FC], BF16, tag="mtile")
                nc.scalar.activation(d1[:], lvl[:, FC * k: FC * (k + 1)],
                                     AF.Copy, bias=0.0, scale=float(1.0 - REL))
                init = 0.0 if k == 0 else env[:, FC * k - 1: FC * k]
                nc.vector.tensor_tensor_scan(
                    env[:, FC * k: FC * (k + 1)], relc[:], d1[:], init,
                    op0=ao.mult, op1=ao.add)
            pr = ps.tile([128, 512], dt, tag="psmm")
            nc.tensor.matmul(pr[:1, :128], env[:, F - 1: F], b_id[:])
            nc.scalar.copy(rowsc[0:1, 1:129], pr[:1, :128])
            nc.vector.tensor_tensor_scan(
                irow[:], relFrow[:], rowsc[0:1, 0:128], 0.0,
                op0=ao.mult, op1=ao.add)
            pc = ps.tile([128, 512], dt, tag="psmm")
            nc.tensor.matmul(pc[:128, :1], irow[:], t_ones[:])
            nc.scalar.copy(icol[:], pc[:128, :1])
            for k in range(8):
                isc = tiny.tile([128, 1], dt, tag="isc")
                nc.vector.tensor_scalar_mul(isc[:], icol[:],
                                            float(REL ** (FC * k)))
                nc.vector.scalar_tensor_tensor(
                    env[:, FC * k: FC * (k + 1)], t_relpow[:], isc[:, 0:1],
                    env[:, FC * k: FC * (k + 1)], op0=ao.mult, op1=ao.add)

            # policy iterations, pipelined: all masks + coefficient tensors
            # come from the previous iterate (Jacobi boundaries — same
            # convergence, validated); the 8 scans then chain serially with
            # current inits.  Act (w1/cf-even) overlaps DVE (is_gt/mul/scan);
            # odd-chunk cf on DVE to balance engine load.
            with tc.tile_pool(name="mp", bufs=8) as mp, \
                 tc.tile_pool(name="cfp", bufs=4) as cfp:
                for it in range(N_ITER):
                    psh = ps.tile([128, 512], dt, tag="psmm")
                    nc.tensor.matmul(psh[:, 0:1], b_ids[:], env[:, F - 1: F])
                    nc.scalar.copy(lanecol[:], psh[:, 0:1])
                    ms, cfs = [], []
                    for k in range(8):
                        base = FC * k
                        m = mp.tile([128, FC], BF16, tag="m")
                        bc = (lanecol[:, 0:1] if k == 0
                              else env[:, base - 1: base])
                        nc.vector.tensor_tensor(
                            m[:, 1:], lvl[:, base + 1: base + FC],
                            env[:, base: base + FC - 1], op=ao.is_gt)
                        nc.vector.tensor_tensor(
                            m[:, 0:1], lvl[:, base: base + 1], bc,
                            op=ao.is_gt)
                        ms.append(m)
                    for k in range(8):
                        base = FC * k
                        m = ms[k]
                        cf = cfp.tile([128, FC], dt, tag="cf")
                        if k % 2 == 0:
                            nc.scalar.activation(cf[:], m[:], AF.Copy,
                                                 bias=float(REL),
                                                 scale=float(ATK - REL))
                        else:
                            nc.vector.tensor_scalar(cf[:], m[:],
                                                    float(ATK - REL),
                                                    float(REL),
                                                    op0=ao.mult, op1=ao.add)
                        cfs.append(cf)
                        # w1 then ct, in place on the mask tile
                        nc.scalar.activation(
                            m[:], m[:], AF.Copy, bias=float(1.0 - REL),
                            scale=float((1.0 - ATK) - (1.0 - REL)))
                        nc.vector.tensor_mul(m[:], m[:],
                                             lvl[:, base: base + FC])
                        # scan for chunk k-1 interleaves with chunk k coeffs
                        if k >= 1:
                            b2 = FC * (k - 1)
                            bcs = (lanecol[:, 0:1] if k == 1
                                   else env[:, b2 - 1: b2])
                            nc.vector.tensor_tensor_scan(
                                env[:, b2: b2 + FC], cfs[k - 1][:],
                                ms[k - 1][:], bcs, op0=ao.mult, op1=ao.add)
                    nc.vector.tensor_tensor_scan(
                        env[:, FC * 7: F], cfs[7][:], ms[7][:],
                        env[:, FC * 7 - 1: FC * 7], op0=ao.mult, op1=ao.add)

            if debug_taps:
                nc.sync.dma_start(
                    dbg_env[:].rearrange("(p f) -> p f", p=128), env[:])
            # gr and y = x * (gr * wdry) (in-place onto yeq); ln(wdry) rides
            # the Exp bias so the dry mix weight costs nothing. g in bf16 (ln
            # sees values >= 1, so 0.4% input rounding is harmless)
            for k in range(8):
                base = FC * k
                g = chk.tile([128, FC], BF16, tag="mtile")
                nc.vector.tensor_scalar(g[:], env[:, base: base + FC],
                                        float(THR), float(1.0 / THR),
                                        op0=ao.max, op1=ao.mult)
                nc.scalar.activation(g[:], g[:], AF.Ln)
                nc.scalar.activation(g[:], g[:], AF.Exp,
                                     bias=t_lnw[:, 0:1], scale=float(GR_EXP))
                nc.vector.tensor_mul(yeq[:, base: base + FC],
                                     yeq[:, base: base + FC], g[:])
        ycomp = yeq

        # ================= Phase Ex: pair exchange.  Owner (cores 4-7, mown=1)
        # processes the FIRST half of its row locally (grid at abs 0 -> the
        # allpass first-epoch quirk lives in its tile 0); partner (cores 0-3,
        # mrecv=1) receives [HS-PAD, NP) and processes the second half.
        sdv = senddram[:].rearrange("(p f) -> p f", p=RL)
        nc.sync.dma_start(sdv[:], ycomp[128 - RL: 128, :])
        if with_collective:
            nc.gpsimd.collective_compute(
                "AllGather", ao.bypass,
                replica_groups=[[0, 4], [1, 5], [2, 6], [3, 7]],
                ins=[senddram[:].opt()],
                outs=[agdram[:].opt()],
            )
        else:
            nc.sync.dma_start(agdram[RN: 2 * RN], senddram[:])
        agt = bbf.tile([128, F], BF16, tag="bigbf")
        agv = agdram[RN: 2 * RN].rearrange("(p f) -> p f", p=RL)
        rinv = ydram[0:RN].rearrange("(p f) -> p f", p=RL)
        with tc.tile_pool(name="mgp", bufs=2) as mgp:
            for k in range(4):
                c0, c1 = FCB * k, FCB * (k + 1)
                nc.sync.dma_start(agt[0:RL, c0:c1], agv[:, c0:c1])
                t1 = mgp.tile([128, FCB], BF16, tag="mg")
                nc.scalar.activation(t1[0:RL, :], ycomp[0:RL, c0:c1],
                                     AF.Copy, scale=t_mown[0:RL, 0:1])
                nc.vector.scalar_tensor_tensor(
                    agt[0:RL, c0:c1], agt[0:RL, c0:c1], t_mrecv[0:RL, 0:1],
                    t1[0:RL, :], op0=ao.mult, op1=ao.add)
                nc.sync.dma_start(rinv[:, c0:c1], agt[0:RL, c0:c1])

        # ================= Phase E: reverb (bf16) over rin [RN]
        with tc.tile_pool(name="rvin", bufs=4) as rvin, \
             tc.tile_pool(name="rvout", bufs=3) as rvout:
            zt = rvin.tile([128, _YTAILC], BF16, tag="rv_ztail")
            nc.gpsimd.memset(zt[:], 0.0)
            nc.sync.dma_start(
                ydram[RN:].rearrange("(p f) -> p f", p=128), zt[:])

            def epoch_filter(src_dram, dst_dram, d, lhsT_t0, lhsT, accum,
                             G=1, in_dt=BF16, out_dt=BF16):
                Tt, cover = _rev_tiles(d)
                packmm = max(1, 512 // d)   # tiles per matmul (pack*d <= 512)

                def do_group(t, g):
                    # tiles t..t+g-1 (t>=1: warmup rows; t==0 solo, no warmup)
                    it_ = rvin.tile([128, g, d], in_dt, tag="rv_in")
                    if t == 0:
                        nc.sync.dma_start(
                            it_[:, 0, :],
                            src_dram[0: 128 * d].rearrange("(e i) -> e i",
                                                           e=128))
                    else:
                        soff = (VEP * t - WEP) * d
                        nc.sync.dma_start(
                            it_[:], bass.AP(tensor=src_dram, offset=soff,
                                            ap=[[d, 128], [VEP * d, g],
                                                [1, d]]))
                    ot = rvout.tile([128, g, d], out_dt, tag="rv_out")
                    lt = lhsT_t0 if t == 0 else lhsT
                    j = 0
                    while j < g:
                        pk = min(packmm, g - j)
                        sub = pk * d
                        offd = 0
                        while offd < d * pk:   # chunk if d > 512
                            s2 = min(512, sub - offd)
                            pe = ps.tile([128, 512], dt, tag="psmm")
                            rhs = (it_[:, j, offd: offd + s2] if pk == 1
                                   else it_[:, j: j + pk, :])
                            o2 = (ot[:, j, offd: offd + s2] if pk == 1
                                  else ot[:, j: j + pk, :])
                            ev = cpeng[(t + j) % 2]
                            if pk == 1:
                                nc.tensor.matmul(pe[:, :s2], lt[:], rhs)
                                ev(o2, pe[:, :s2])
                            else:
                                nc.tensor.matmul(
                                    pe[:, :sub].rearrange(
                                        "p (j i) -> p j i", j=pk), lt[:], rhs)
                                ev(o2, pe[:, :sub].rearrange(
                                    "p (j i) -> p j i", j=pk))
                            offd += s2
                        j += pk
                    if t == 0:
                        rows = ot[0:VEP, 0, :]
                        dst = dst_dram[0: VEP * d].rearrange("(e i) -> e i",
                                                             e=VEP)
                        eng = nc.gpsimd if accum else nc.sync
                        eng.dma_start(dst, rows,
                                      **({"accum_op": ao.add} if accum else {}))
                    else:
                        if not accum:
                            dap = bass.AP(tensor=dst_dram, offset=VEP * t * d,
                                          ap=[[d, VEP], [VEP * d, g], [1, d]])
                            nc.sync.dma_start(dap, ot[WEP:128, :, :])
                        else:
                            # SWDGE ring holds ~1024 descriptors; keep each
                            # accumulating DMA under ~700 (6*116 descs)
                            j0 = 0
                            while j0 < g:
                                gg = min(6, g - j0)
                                dap = bass.AP(
                                    tensor=dst_dram,
                                    offset=(VEP * (t + j0)) * d,
                                    ap=[[d, VEP], [VEP * d, gg], [1, d]])
                                nc.gpsimd.dma_start(
                                    dap, ot[WEP:128, j0: j0 + gg, :],
                                    accum_op=ao.add)
                                j0 += gg

                do_group(0, 1)
                t = 1
                while t < Tt:
                    g = min(G, Tt - t)
                    do_group(t, g)
                    t += g

            combs = sorted(COMB_DELAYS, key=lambda d: -_rev_tiles(d)[1])
            for ci, d in enumerate(combs):
                epoch_filter(ydram, wetdram, d, b_lcT, b_lcT,
                             accum=(ci > 0 and not no_accum), out_dt=dt)
            epoch_filter(wetdram, apdram, AP_DELAYS[0], t_laqT, t_laT, False,
                         G=8, in_dt=dt)
            if _AP0_SIZE > _AP0_COVER:
                gap = _AP0_SIZE - _AP0_COVER
                ztg = rvin.tile([128, 1, 2030], BF16, tag="rv_in")
                nc.gpsimd.memset(ztg[:], 0.0)
                nc.sync.dma_start(
                    apdram[_AP0_COVER:].rearrange("(o f) -> o f", o=1),
                    ztg[0:1, 0, 0:gap])
            epoch_filter(apdram, wet2dram, AP_DELAYS[1], b_laqT2, b_laT2,
                         False, G=24)

        # ============= Phase F: place the wet half + dry mix + collective +
        # clip.  Both candidate wet windows are loaded (owner valid at rin
        # [0, HS), partner at [PAD, PAD+HS)); `placed` zeroes the wrong one.
        # ycomp is already wdry-scaled and wet2 carries WET/(1-WET), so the
        # blend is one masked add.
        with tc.tile_pool(name="finp", bufs=2) as finp:
            w2own = wet2dram[0:HS].rearrange("(p f) -> p f", p=HSL)
            w2par = wet2dram[PAD: PAD + HS].rearrange("(p f) -> p f", p=HSL)
            mdv = mixdram[0:NP].rearrange("(p f) -> p f", p=128)
            for k in range(4):
                c0, c1 = FCB * k, FCB * (k + 1)
                nc.sync.dma_start(agt[0:HSL, c0:c1], w2own[:, c0:c1])
                nc.sync.dma_start(agt[HSL:128, c0:c1], w2par[:, c0:c1])
                nc.vector.scalar_tensor_tensor(
                    ycomp[:, c0:c1], agt[:, c0:c1], t_placed[:, 0:1],
                    ycomp[:, c0:c1], op0=ao.mult, op1=ao.add)
                nc.sync.dma_start(mdv[:, c0:c1], ycomp[:, c0:c1])
            if with_collective:
                nc.gpsimd.collective_compute(
                    "ReduceScatter", ao.add,
                    replica_groups=[[0, 2, 4, 6], [1, 3, 5, 7]],
                    ins=[mixdram[0:NP].opt()],
                    outs=[rsdram.ap().opt()],
                )
            else:
                nc.sync.dma_start(rsdram[:], mixdram[0:CH])
            rs2 = rsdram[:].rearrange("(p f) -> p f", p=128)   # [128, 2816]
            o2 = out[:].rearrange("(p f) -> p f", p=128)
            for hh in range(2):
                oc = finp.tile([128, FC], BF16, tag="f_oc")
                ocf = finp.tile([128, FC], dt, tag="f_ocf")
                nc.sync.dma_start(oc[:], rs2[:, FC * hh: FC * (hh + 1)])
                nc.vector.tensor_scalar(ocf[:], oc[:], float(-CEIL), float(CEIL),
                                        op0=ao.max, op1=ao.min)
                nc.sync.dma_start(o2[:, FC * hh: FC * (hh + 1)], ocf[:])

    nc.compile()
    return nc


# ============================================================== host wrapper
_CACHE = {}

def _get_program():
    if "nc" not in _CACHE:
        _CACHE["nc"] = build_program()
    return _CACHE["nc"]


def _host_consts():
    if "consts" in _CACHE:
        return _CACHE["consts"]
    ident = np.eye(128, dtype=np.float32)
    Lc = np.ascontiguousarray((0.25 * _epoch_matrix_comb(FB)).T.astype(np.float32))
    SC = WET / (1.0 - WET)     # wwet/wdry, folded into the ap81 matrices
    La = np.ascontiguousarray(_epoch_matrix_ap(FB).T.astype(np.float32))
    Laq = np.ascontiguousarray(_epoch_matrix_ap(FB, quirk=True).T.astype(np.float32))
    La2 = np.ascontiguousarray((SC * _epoch_matrix_ap(FB)).T.astype(np.float32))
    Laq2 = np.ascontiguousarray(
        (SC * _epoch_matrix_ap(FB, quirk=True)).T.astype(np.float32))
    relpow = np.ascontiguousarray(np.broadcast_to(
        (REL ** (np.arange(FC, dtype=np.float64) + 1.0)).astype(np.float32),
        (128, FC)))
    eqc = {}
    for t in range(4):
        T, zmT_, zc_ = _track_eq_consts(t)
        eqc[t] = (np.ascontiguousarray(T.T.astype(np.float32)),
                  np.ascontiguousarray(zmT_.astype(np.float32)),
                  np.ascontiguousarray(zc_.astype(np.float32)))
    _CACHE["consts"] = (ident, Lc, La, Laq, La2, Laq2, relpow, eqc)
    return _CACHE["consts"]


def kernel(tracks, volumes, pans):
    tracks = np.ascontiguousarray(np.asarray(tracks, np.float32))
    volumes = np.asarray(volumes, np.float32)
    pans = np.asarray(pans, np.float32)

    angle = (pans.astype(np.float64) + 1.0) * 0.25 * math.pi
    lg, rg = np.cos(angle), np.sin(angle)
    ident, Lc, La, Laq, La2, Laq2, relpow, eqc = _host_consts()

    in_maps = []
    for core in range(8):
        t, ch = core // 2, core % 2
        is_owner = core >= 4      # owns a reverb row, processes its 1st half
        xpad = np.zeros(NP, np.float32)
        xpad[:N] = tracks[t, ch]
        thT_np, zm_np, zc_np = eqc[t]
        w = float(volumes[t]) * float(lg[t] if ch == 0 else rg[t])
        has_rev = t >= 2
        w_dry = w * (1.0 - WET) if has_rev else w
        placed = np.zeros((128, 1), np.float32)
        if is_owner:
            placed[0:HSL] = 1.0
        else:
            placed[HSL:128] = 1.0
        in_maps.append({
            "x": xpad, "thT": thT_np, "zm": zm_np, "zc": zc_np,
            "ident": ident, "lcT": Lc, "laT": La,
            "laqT": Laq if is_owner else La,
            "laT2": La2, "laqT2": Laq2 if is_owner else La2,
            "relpow": relpow,
            "lnwdry": np.full((128, 1), math.log(max(w_dry, 1e-30)),
                              np.float32),
            "mown": np.full((128, 1), 1.0 if is_owner else 0.0, np.float32),
            "mrecv": np.full((128, 1), 0.0 if is_owner else 1.0, np.float32),
            "placed": placed,
        })

    nc = _get_program()
    res = run_bass_kernel_spmd(nc, in_maps, list(range(8)))

    outp = np.zeros((2, N), np.float32)
    for ch in range(2):
        full = np.concatenate([res.results[2 * q + ch]["out"] for q in range(4)])
        outp[ch] = full[:N]
    return outp

